# revision 1
# baseline (speedup 1.0000x reference)
"""CP tensor product ('uvu' connection) kernel for Trainium2, SPMD over 8 NeuronCores.

Math per batch element b (reassociation of the reference einsum):
  q   = x2[b] @ w[b].T               (16, 64)  per-b PE matmuls (M=32 pair-junk trick)
  t1  = A.T @ x1[b]                  (64, 64)  batched N=512 matmuls, A stationary
  t3  = B.T @ q                      (64, 64)  batched N=512 matmuls, B stationary
  m   = t1 * t3                                DVE tensor_mul (t1 from PSUM, t3 from SBUF)
  out = (C.T).T @ m                  (16, 64)  batched N=512 matmuls, C.T stationary

This equals the reference out = einsum('cr,bro->bco', C, (x1A) * ((x2B)@w^T))
because einsum('brv,bov->bro', x2@B, w) == B.T @ (x2 @ w.T) per b.

Sharding: batch (32768) split evenly across the 8 cores (data parallel);
A/B/C replicated. All compute fp32; fp32 accumulation in PSUM.

Per-core layout (per 128-b block; sub-block s = (b//32)%4; octet j = (b//8)%4):
  x1[b]  at x1_t[32s : 32s+16,  512*j + 64*(b%8) : +64]        (d, o)
  wT[b]  at wT_t[32s : 32s+32,  64*(b%32) : +64]               (v, o)  via DVE 32x32
  x2T[b] at x2T_t[32s : 32s+32, 32*((b%32)//2)+16*(b%2) : +16] (v, j)  block transpose
  q[b]   at q_ps[32j : 32j+16, 64*(b%8) : +64]  (+16 junk rows from M=32 trick)
  t1/t3  (128,1024) PSUM tiles: octet j -> partitions 64*(j%2), free 512*(j//2)
  out[b] at o_ps[32j : 32j+16, 64*(b%8) : +64]  (+16 junk rows)
"""
import time
import numpy as np
from contextlib import ExitStack

import jax
from jax.experimental.shard_map import shard_map
from jax.sharding import Mesh, PartitionSpec, NamedSharding

import concourse.bass as bass
import concourse.bacc as bacc
import concourse.tile as tile
import concourse.mybir as mybir
from concourse._compat import with_exitstack
from concourse.bass2jax import _bass_exec_p, install_neuronx_cc_hook, partition_id_tensor

F32 = mybir.dt.float32

NCORES = 8
BATCH = 32768
B_LOCAL = BATCH // NCORES
D = 16
CH1 = 64
CH2 = 32
RANK = 64
BLK = 128


def _emit(ctx: ExitStack, tc: tile.TileContext, outs, ins, b_local: int):
    nc = tc.nc
    (out_d,) = outs
    (x1_d, x2_d, w_d, a_d, b_d, ct_d) = ins
    nblk = b_local // BLK

    const = ctx.enter_context(tc.tile_pool(name="const", bufs=1))
    A_sb = const.tile([128, CH1], F32)
    B_sb = const.tile([128, RANK], F32)
    CT_sb = const.tile([128, 32], F32)
    for rp in range(4):
        nc.sync.dma_start(A_sb[32 * rp:32 * rp + 16, :], a_d[:, :])
        nc.sync.dma_start(B_sb[32 * rp:32 * rp + 16, :], b_d[:, :])
    for rp2 in (0, 64):
        for cj in (0, 16):
            nc.sync.dma_start(CT_sb[rp2:rp2 + 64, cj:cj + 16], ct_d[:, :])

    x1_pool = ctx.enter_context(tc.tile_pool(name="x1", bufs=2))
    x2_pool = ctx.enter_context(tc.tile_pool(name="x2", bufs=2))
    w_pool = ctx.enter_context(tc.tile_pool(name="w", bufs=2))
    x2T_pool = ctx.enter_context(tc.tile_pool(name="x2T", bufs=2))
    wT_pool = ctx.enter_context(tc.tile_pool(name="wT", bufs=2))
    qsb_pool = ctx.enter_context(tc.tile_pool(name="qsb", bufs=2))
    m_pool = ctx.enter_context(tc.tile_pool(name="m", bufs=4))
    osb_pool = ctx.enter_context(tc.tile_pool(name="osb", bufs=3))
    pq = ctx.enter_context(tc.tile_pool(name="pq", bufs=1, space="PSUM"))
    pt = ctx.enter_context(tc.tile_pool(name="pt", bufs=6, space="PSUM"))
    po = ctx.enter_context(tc.tile_pool(name="po", bufs=1, space="PSUM"))
    t3sb_pool = ctx.enter_context(tc.tile_pool(name="t3sb", bufs=2))

    for blk in range(nblk):
        b0 = blk * BLK
        x1_t = x1_pool.tile([128, 2048], F32)
        for rp in range(4):
            src = x1_d[b0 + 32 * rp:b0 + 32 * rp + 32, :, :].rearrange("b d o -> d b o")
            dst = x1_t[32 * rp:32 * rp + 16, :].rearrange("p (b o) -> p b o", o=64)
            nc.sync.dma_start(dst, src)
        x2_t = x2_pool.tile([128, 512], F32)
        for g in range(4):
            for pr in range(2):
                src = x2_d[b0 + 32 * g + pr:b0 + 32 * (g + 1):2, :, :].rearrange("c j v -> j c v")
                dst = x2_t[32 * g + 16 * pr:32 * g + 16 * pr + 16, :].rearrange("p (c v) -> p c v", v=32)
                nc.sync.dma_start(dst, src)
        w_t = w_pool.tile([128, 2048], F32)
        for g in range(4):
            for h in range(2):
                src = w_d[b0 + 32 * g:b0 + 32 * g + 32, 32 * h:32 * h + 32, :].rearrange("s p v -> p s v")
                dst = w_t[32 * g:32 * g + 32, :].rearrange("p (s hv) -> p s hv", hv=64)[:, :, 32 * h:32 * h + 32]
                nc.sync.dma_start(dst, src)

        x2T_t = x2T_pool.tile([128, 544], F32)
        nc.vector.transpose(x2T_t[:, 0:512], x2_t[:])
        nc.vector.memset(x2T_t[:, 512:544], 0.0)
        wT_t = wT_pool.tile([128, 2048], F32)
        nc.vector.transpose(wT_t[:], w_t[:])

        q_sb = qsb_pool.tile([128, 2048], F32)
        for s in range(4):
            bs = b0 + 32 * s
            q_ps = pq.tile([128, 512], F32)
            for k in range(8):
                for j in range(4):
                    bb = 8 * j + k
                    x = 32 * (bb // 2) + 16 * (bb % 2)
                    nc.tensor.matmul(
                        q_ps[32 * j:32 * j + 32, 64 * k:64 * k + 64],
                        x2T_t[32 * s:32 * s + 32, x:x + 32],
                        wT_t[32 * s:32 * s + 32, 64 * bb:64 * bb + 64],
                        tile_position=(32 * s, 32 * j),
                    )
            nc.scalar.copy(q_sb[:, 512 * s:512 * (s + 1)], q_ps[:])

            t1a = pt.tile([128, 512], F32, tag="t")
            t1b = pt.tile([128, 512], F32, tag="t")
            t3a = pt.tile([128, 512], F32, tag="t")
            t3b = pt.tile([128, 512], F32, tag="t")
            t1x = [t1a, t1a, t1b, t1b]
            t3x = [t3a, t3a, t3b, t3b]
            for j in range(4):
                cp = 64 * (j % 2)
                nc.tensor.matmul(
                    t1x[j][cp:cp + 64, :],
                    A_sb[32 * s:32 * s + 16, :],
                    x1_t[32 * s:32 * s + 16, 512 * j:512 * (j + 1)],
                    tile_position=(32 * s, cp),
                )
                nc.tensor.matmul(
                    t3x[j][cp:cp + 64, :],
                    B_sb[32 * j:32 * j + 16, :],
                    q_sb[32 * j:32 * j + 16, 512 * s:512 * (s + 1)],
                    tile_position=(32 * j, cp),
                )

            t3_sb = t3sb_pool.tile([128, 1024], F32)
            nc.scalar.copy(t3_sb[:, 0:512], t3a[:])
            nc.scalar.copy(t3_sb[:, 512:1024], t3b[:])
            m_t = m_pool.tile([128, 1024], F32)
            nc.vector.tensor_mul(m_t[:, 0:512], t1a[:], t3_sb[:, 0:512])
            nc.vector.tensor_mul(m_t[:, 512:1024], t1b[:], t3_sb[:, 512:1024])

            o_ps = po.tile([128, 512], F32)
            for j in range(4):
                rp2 = 64 * (j % 2)
                nc.tensor.matmul(
                    o_ps[32 * j:32 * j + 32, :],
                    CT_sb[rp2:rp2 + 64, :],
                    m_t[rp2:rp2 + 64, 512 * (j // 2):512 * (j // 2) + 512],
                    tile_position=(rp2, 32 * j),
                )
            o_sb = osb_pool.tile([128, 512], F32)
            nc.scalar.copy(o_sb[:], o_ps[:])
            for j in range(4):
                dst = out_d[bs + 8 * j:bs + 8 * j + 8, :, :].rearrange("k c o -> c k o")
                src = o_sb[32 * j:32 * j + 16, :].rearrange("p (k o) -> p k o", o=64)
                nc.sync.dma_start(dst, src)


@with_exitstack
def _cp_kernel(ctx, tc, outs, ins, b_local):
    _emit(ctx, tc, outs, ins, b_local)


def build_nc(b_local: int = B_LOCAL):
    nc = bacc.Bacc("TRN2", target_bir_lowering=False, debug=False)
    x1_d = nc.dram_tensor("x1", [b_local, D, CH1], F32, kind="ExternalInput").ap()
    x2_d = nc.dram_tensor("x2", [b_local, D, CH2], F32, kind="ExternalInput").ap()
    w_d = nc.dram_tensor("w", [b_local, CH1, CH2], F32, kind="ExternalInput").ap()
    a_d = nc.dram_tensor("a", [D, RANK], F32, kind="ExternalInput").ap()
    b_d = nc.dram_tensor("b", [D, RANK], F32, kind="ExternalInput").ap()
    ct_d = nc.dram_tensor("ct", [RANK, D], F32, kind="ExternalInput").ap()
    out_d = nc.dram_tensor("out", [b_local, D, CH1], F32, kind="ExternalOutput").ap()
    with tile.TileContext(nc, trace_sim=False) as tc:
        _cp_kernel(tc, [out_d], [x1_d, x2_d, w_d, a_d, b_d, ct_d], b_local)
    nc.compile()
    return nc


class _SpmdRunner:
    """Persistent jitted SPMD executor over the 8 NeuronCores."""

    def __init__(self, nc, n_cores=NCORES):
        install_neuronx_cc_hook()
        self.nc = nc
        self.n_cores = n_cores
        pid_name = nc.partition_id_tensor.name if nc.partition_id_tensor else None

        in_names, out_names, out_avals, zero_outs = [], [], [], []
        for alloc in nc.m.functions[0].allocations:
            if not isinstance(alloc, mybir.MemoryLocationSet):
                continue
            name = alloc.memorylocations[0].name
            if alloc.kind == "ExternalInput":
                if name != pid_name:
                    in_names.append(name)
            elif alloc.kind == "ExternalOutput":
                out_names.append(name)
                shape = tuple(alloc.tensor_shape)
                dtype = mybir.dt.np(alloc.dtype)
                out_avals.append(jax.core.ShapedArray(shape, dtype))
                zero_outs.append(np.zeros(shape, dtype))
        self.in_names, self.out_names = in_names, out_names
        self.out_avals, self.zero_outs = out_avals, zero_outs
        n_params = len(in_names)
        all_names = tuple(in_names + out_names + ([pid_name] if pid_name else []))

        def _body(*args):
            operands = list(args)
            if pid_name is not None:
                operands.append(partition_id_tensor())
            outs = _bass_exec_p.bind(
                *operands,
                out_avals=tuple(out_avals),
                in_names=all_names,
                out_names=tuple(out_names),
                lowering_input_output_aliases=(),
                sim_require_finite=True,
                sim_require_nnan=True,
                nc=nc,
            )
            return tuple(outs)

        devices = jax.devices()[:n_cores]
        self.mesh = Mesh(np.asarray(devices), ("core",))
        self.sharding = NamedSharding(self.mesh, PartitionSpec("core"))
        n_out = len(out_names)
        donate = tuple(range(n_params, n_params + n_out))
        self.jitted = jax.jit(
            shard_map(_body, mesh=self.mesh,
                      in_specs=(PartitionSpec("core"),) * (n_params + n_out),
                      out_specs=(PartitionSpec("core"),) * n_out,
                      check_rep=False),
            donate_argnums=donate, keep_unused=True,
        )

    def stage_inputs(self, in_maps):
        per_core = [[np.asarray(m[name]) for name in self.in_names] for m in in_maps]
        concat = [np.concatenate([per_core[c][i] for c in range(self.n_cores)], axis=0)
                  for i in range(len(self.in_names))]
        return [jax.device_put(a, self.sharding) for a in concat]

    def stage_zeros(self):
        zs = [np.zeros((self.n_cores * z.shape[0], *z.shape[1:]), z.dtype)
              for z in self.zero_outs]
        return [jax.device_put(z, self.sharding) for z in zs]

    def run(self, dev_inputs, dev_zeros=None):
        if dev_zeros is None:
            dev_zeros = self.stage_zeros()
        outs = self.jitted(*dev_inputs, *dev_zeros)
        jax.block_until_ready(outs)
        return outs

    def unshard_out(self, outs):
        i = self.out_names.index("out")
        a = np.asarray(outs[i])
        return a  # already (n_cores*b_local, D, CH1) stacked along axis 0


_RUNNER = None


def _get_runner():
    global _RUNNER
    if _RUNNER is None:
        nc = build_nc(B_LOCAL)
        _RUNNER = _SpmdRunner(nc, NCORES)
    return _RUNNER


def kernel(x1, x2, w, A, B, C):
    """Full-input entry point. Shards batch across 8 NeuronCores, runs the
    Bass kernel, gathers the full output (32768, 16, 64) float32."""
    runner = _get_runner()
    x1 = np.ascontiguousarray(np.asarray(x1, dtype=np.float32))
    x2 = np.ascontiguousarray(np.asarray(x2, dtype=np.float32))
    w = np.ascontiguousarray(np.asarray(w, dtype=np.float32))
    A = np.ascontiguousarray(np.asarray(A, dtype=np.float32))
    B = np.ascontiguousarray(np.asarray(B, dtype=np.float32))
    CT = np.ascontiguousarray(np.asarray(C, dtype=np.float32).T)

    bl = x1.shape[0] // NCORES
    in_maps = []
    for c in range(NCORES):
        sl = slice(c * bl, (c + 1) * bl)
        in_maps.append({"x1": x1[sl], "x2": x2[sl], "w": w[sl],
                        "a": A, "b": B, "ct": CT})
    dev_in = runner.stage_inputs(in_maps)
    outs = runner.run(dev_in)
    return runner.unshard_out(outs)



# revision 30
# speedup vs baseline: 10.3120x; 10.3120x over previous
"""CP tensor product ('uvu' connection) kernel for Trainium2, SPMD over 8
NeuronCores.

Math per batch element b (decomposed with b = 4g + s, s in 0..3, s = 2H + s2):
  q[b,j,o]  = sum_v x2[b,j,v] w[b,o,v]          (16,64) per-b
  t1[b,r,o] = sum_d A[d,r] x1[b,d,o]            (64,64)
  t3[b,r,o] = sum_j B[j,r] q[b,j,o]             (64,64)
  out[b,c,o]= sum_r C[c,r] t1[b,r,o] t3[b,r,o]  (16,64)

Design (driven by the TimelineSim cost model, where a matmul costs
out_free_cols * 0.42ns at 1 cycle/row for bf16 and contraction size is free):
every matmul packs 128 output partitions by pairing batch elements (s2) with
static block-diagonal stationaries (a2d/bsel/c2), so PE time ~= total output
elements / 128. The per-b q contraction uses per-group block-diagonal x2
stationaries streamed pre-zero-padded from DRAM (LdWeights is free). All
inputs are bf16 (host-converted); PSUM accumulates fp32.

Hardware constraints shaping the pipeline:
  - DVE/Act ops may read at most ONE PSUM operand -> t3 is staged PSUM->SBUF
    (bf16) by the Act engine, then DVE multiplies t1(PSUM) * t3s(SBUF).
  - GPSIMD cannot access PSUM at all -> Act/DVE carry all staging; gpsimd
    only issues the output DMA queue.
  - 8 PSUM banks total: q[1] + t1[2x 1024-col tiles = 4] + t3[2] + out[1].
  - DMA: >=512B contiguous per descriptor, ~1MB per instruction to hide the
    per-instruction ~1.6us overhead; all transfers share one 360B/ns device.

Per-core DRAM layouts (g = group of 4 b's, chunk t = 8 g's, pair = 2 chunks,
super-chunk sc = 8 chunks; local batch 4096):
  x1h  [64,65536]  row 16s+d  (s<2 | s>=2 duplicated rows 32-63.. see pack),
                   col 64g+o : x1[4g+s,d,o]  (rows 0-31 H=0, 32-63 H=1)
  wh   [128,65536] row 32s+v,  col 64g+o : w[4g+s,o,v]
  x2bd [128,65536] row 32s+v,  col 64g+16s'+j : x2[4g+s,j,v] iff s==s', else 0
  a2d  [64,128]    blockdiag2(A) over (s2,d)x(s2,r); rows 32-63 duplicate
  bsel [128,256]   col block H: B[j,r] delta(s,2H+s2); rows 64-127 duplicate
  c2   [128,32]    blockdiag2(C^T) over (s2,r)x(s2,c)
  outh [128,32768] row 32k+16s2+c with k=2*(t%2)+H, col 512*(t//2)+64*(g%8)+o
"""
import numpy as np
from contextlib import ExitStack

import jax
from jax.experimental.shard_map import shard_map
from jax.sharding import Mesh, PartitionSpec, NamedSharding

import concourse.bass as bass
import concourse.bacc as bacc
import concourse.tile as tile
import concourse.mybir as mybir
from concourse._compat import with_exitstack
from concourse.bass2jax import _bass_exec_p, install_neuronx_cc_hook, partition_id_tensor

F32 = mybir.dt.float32
BF16 = mybir.dt.bfloat16
NPBF16 = mybir.dt.np(BF16)

NCORES = 8
BATCH = 32768
B_LOCAL = BATCH // NCORES   # 4096
NG = B_LOCAL // 4           # 1024 groups of 4 b's
NCHUNK = NG // 8            # 128 chunks (8 groups each)
NPAIR = NCHUNK // 2         # 64 chunk-pairs
NSC = 16                    # super-chunks (8 chunks each)
PAIRS_PER_SC = NPAIR // NSC

# Software-pipeline depths (tuned against TimelineSim):
LAG = 4        # final matmuls trail the t-stage by LAG chunks
QLEAD = 1      # q production leads consumption by QLEAD+1 pairs
SLAB_BUFS = 3  # input half-slab buffers per pool
SC_LEAD = 1    # super-chunk load lead


def _emit(ctx: ExitStack, tc: tile.TileContext, outs, ins):
    nc = tc.nc
    (outh,) = outs
    (x1h, wh, x2bd, a2d, bsel, c2) = ins

    const = ctx.enter_context(tc.tile_pool(name="const", bufs=1))
    a2_sb = const.tile([64, 128], BF16)
    bsel_sb = const.tile([128, 256], BF16)
    c2_sb = const.tile([128, 32], BF16)
    nc.sync.dma_start(a2_sb[:], a2d[:, :])
    nc.sync.dma_start(bsel_sb[:], bsel[:, :])
    nc.sync.dma_start(c2_sb[:], c2[:, :])

    x1_pool = ctx.enter_context(tc.tile_pool(name="x1", bufs=SLAB_BUFS))
    wh_pool = ctx.enter_context(tc.tile_pool(name="wh", bufs=SLAB_BUFS))
    x2_pool = ctx.enter_context(tc.tile_pool(name="x2", bufs=SLAB_BUFS))
    qsb_pool = ctx.enter_context(tc.tile_pool(name="qsb", bufs=max(2, QLEAD + 1)))
    msb_pool = ctx.enter_context(tc.tile_pool(name="msb", bufs=LAG + 1))
    t3sb_pool = ctx.enter_context(tc.tile_pool(name="t3sb", bufs=2))
    osb_pool = ctx.enter_context(tc.tile_pool(name="osb", bufs=2))
    pq = ctx.enter_context(tc.tile_pool(name="pq", bufs=1, space="PSUM"))
    pt1 = ctx.enter_context(tc.tile_pool(name="pt1", bufs=2, space="PSUM"))
    pt3 = ctx.enter_context(tc.tile_pool(name="pt3", bufs=1, space="PSUM"))
    po = ctx.enter_context(tc.tile_pool(name="po", bufs=1, space="PSUM"))

    slabs = {}   # sc -> (x1_halves, w_halves, x2_halves, q_tile)
    m_of = {}    # chunk -> m_sb tile [128,1024] (cols 0:512 H=0, 512: H=1)
    o_of = {}    # pair -> out psum tile
    osb_of = {}  # sc -> out sbuf slab

    def load_sc(sc):
        # two half-slabs per tensor so compute can start after half a load
        x1_h, w_h, x2_h = [], [], []
        for h in range(2):
            cw = slice(4096 * sc + 2048 * h, 4096 * sc + 2048 * (h + 1))
            x1_t = x1_pool.tile([64, 2048], BF16, name="x1s")
            nc.sync.dma_start(x1_t[:], x1h[:, cw])
            w_t = wh_pool.tile([128, 2048], BF16, name="ws")
            nc.sync.dma_start(w_t[:], wh[:, cw])
            x2_t = x2_pool.tile([128, 2048], BF16, name="x2s")
            nc.sync.dma_start(x2_t[:], x2bd[:, cw])
            x1_h.append(x1_t); w_h.append(w_t); x2_h.append(x2_t)
        q_t = qsb_pool.tile([128, 2048], BF16, name="qs")
        slabs[sc] = (x1_h, w_h, x2_h, q_t)

    def emit_q(pair):
        # 16 q matmuls (one per group) for both chunks of `pair` into one
        # [128,512] psum bank (chunk parity -> partition offset 0/64), then
        # one Act copy stages it to SBUF bf16.
        (x1_h, w_h, x2_h, q_t) = slabs[pair // PAIRS_PER_SC]
        ps = pq.tile([128, 512], F32, name="qps")
        for tp in range(2):            # chunk t = 2*pair + tp
            t = 2 * pair + tp
            for gg in range(8):        # group g = 8*t + gg
                gcol = 64 * ((t % 8) * 8 + gg)   # col offset within slab
                nc.tensor.matmul(
                    ps[64 * tp:64 * tp + 64, 64 * gg:64 * gg + 64],
                    x2_h[gcol // 2048][:, gcol % 2048:gcol % 2048 + 64],
                    w_h[gcol // 2048][:, gcol % 2048:gcol % 2048 + 64],
                    tile_position=(0, 64 * tp),
                )
        pb = pair % PAIRS_PER_SC
        nc.scalar.copy(q_t[:, 512 * pb:512 * (pb + 1)], ps[:])

    def emit_tstage(t):
        # t1/t3 matmuls (H-interleaved), Act stages t3 to SBUF bf16, DVE
        # multiplies m = t1 * t3s.
        (x1_h, w_h, x2_h, q_t) = slabs[t // 8]
        x1_t = x1_h[(t % 8) // 4]
        tp = t % 2
        cw0 = 512 * ((t % 8) % 4)
        colw = slice(cw0, cw0 + 512)
        qcw = slice(512 * ((t % 8) // 2), 512 * ((t % 8) // 2) + 512)
        t1 = pt1.tile([128, 1024], F32, name="t1ps")
        t3 = pt3.tile([128, 1024], F32, name="t3ps")
        for H in range(2):
            nc.tensor.matmul(
                t1[:, 512 * H:512 * H + 512],
                a2_sb[32 * H:32 * H + 32, :],
                x1_t[32 * H:32 * H + 32, colw],
                tile_position=(32 * H, 0),
            )
            nc.tensor.matmul(
                t3[:, 512 * H:512 * H + 512],
                bsel_sb[64 * tp:64 * tp + 64, 128 * H:128 * (H + 1)],
                q_t[64 * tp:64 * tp + 64, qcw],
                tile_position=(64 * tp, 0),
            )
        t3s = t3sb_pool.tile([128, 1024], BF16, name="t3s")
        nc.scalar.copy(t3s[:], t3[:])
        m = msb_pool.tile([128, 1024], BF16, name="ms")
        nc.vector.tensor_mul(m[:], t1[:], t3s[:])
        m_of[t] = m

    def emit_finals(t):
        # C contraction for chunk t into the pair's out psum bank (4 stacked
        # 32-partition strips), staged out and DMAed per super-chunk.
        pair = t // 2
        tp = t % 2
        if tp == 0:
            o_of[pair] = po.tile([128, 512], F32, name="ops")
        o_ps = o_of[pair]
        m = m_of.pop(t)
        for H in range(2):
            k = 2 * tp + H
            nc.tensor.matmul(
                o_ps[32 * k:32 * k + 32, :], c2_sb[:],
                m[:, 512 * H:512 * H + 512],
                tile_position=(0, 32 * k),
            )
        if tp == 1:
            sc = pair // PAIRS_PER_SC
            if sc not in osb_of:
                osb_of[sc] = osb_pool.tile([128, 2048], BF16, name="osb")
            pp = pair % PAIRS_PER_SC
            odst = osb_of[sc][:, 512 * pp:512 * (pp + 1)]
            ops = o_of.pop(pair)
            if pair % 2 == 0:
                nc.vector.tensor_scalar_mul(odst, ops[:], 1.0)
            else:
                nc.scalar.copy(odst, ops[:])
            if pp == PAIRS_PER_SC - 1:
                nc.sync.dma_start(outh[:, 2048 * sc:2048 * (sc + 1)],
                                  osb_of.pop(sc)[:])

    for k in range(SC_LEAD + 1):
        load_sc(k)
    for p in range(QLEAD + 1):
        emit_q(p)
    for t in range(NCHUNK + LAG):
        if t < NCHUNK:
            if t % 8 == 0 and t // 8 + SC_LEAD + 1 < NSC:
                load_sc(t // 8 + SC_LEAD + 1)
            if t % 2 == 0 and t // 2 + QLEAD + 1 < NPAIR:
                emit_q(t // 2 + QLEAD + 1)
            emit_tstage(t)
            if t >= LAG:
                emit_finals(t - LAG)
        else:
            emit_finals(t - LAG)


@with_exitstack
def _cp_kernel(ctx, tc, outs, ins):
    _emit(ctx, tc, outs, ins)


def build_nc():
    nc = bacc.Bacc("TRN2", target_bir_lowering=False, debug=False)
    x1h = nc.dram_tensor("x1h", [64, 65536], BF16, kind="ExternalInput").ap()
    wh = nc.dram_tensor("wh", [128, 65536], BF16, kind="ExternalInput").ap()
    x2bd = nc.dram_tensor("x2bd", [128, 65536], BF16, kind="ExternalInput").ap()
    a2d = nc.dram_tensor("a2d", [64, 128], BF16, kind="ExternalInput").ap()
    bsel = nc.dram_tensor("bsel", [128, 256], BF16, kind="ExternalInput").ap()
    c2 = nc.dram_tensor("c2", [128, 32], BF16, kind="ExternalInput").ap()
    outh = nc.dram_tensor("outh", [128, 32768], BF16, kind="ExternalOutput").ap()
    with tile.TileContext(nc, trace_sim=False) as tc:
        _cp_kernel(tc, [outh], [x1h, wh, x2bd, a2d, bsel, c2])
    nc.compile()
    return nc


def pack_inputs(x1, x2, w, A, B, C):
    """Host-side: full fp32 arrays -> per-core bf16 packed arrays (list of
    dicts keyed by dram tensor name)."""
    x1 = np.asarray(x1, np.float32)
    x2 = np.asarray(x2, np.float32)
    w = np.asarray(w, np.float32)
    A = np.asarray(A, np.float32)
    B = np.asarray(B, np.float32)
    C = np.asarray(C, np.float32)

    a2d = np.zeros((64, 128), np.float32)
    for s2 in range(2):
        a2d[16 * s2:16 * s2 + 16, 64 * s2:64 * s2 + 64] = A
    a2d[32:64] = a2d[0:32]
    bsel = np.zeros((128, 256), np.float32)
    for H in range(2):
        for s2 in range(2):
            s = 2 * H + s2
            bsel[16 * s:16 * s + 16, 128 * H + 64 * s2:128 * H + 64 * s2 + 64] = B
    bsel[64:128] = bsel[0:64]
    c2 = np.zeros((128, 32), np.float32)
    for s2 in range(2):
        c2[64 * s2:64 * s2 + 64, 16 * s2:16 * s2 + 16] = C.T

    a2d = a2d.astype(NPBF16)
    bsel = bsel.astype(NPBF16)
    c2 = c2.astype(NPBF16)

    in_maps = []
    for cidx in range(NCORES):
        sl = slice(cidx * B_LOCAL, (cidx + 1) * B_LOCAL)
        x1c = x1[sl].reshape(NG, 4, 16, 64)          # [g,s,d,o]
        x1h = np.ascontiguousarray(
            x1c.transpose(1, 2, 0, 3)).reshape(64, 65536).astype(NPBF16)
        wc = w[sl].reshape(NG, 4, 64, 32)            # [g,s,o,v]
        wh = np.ascontiguousarray(
            wc.transpose(1, 3, 0, 2)).reshape(128, 65536).astype(NPBF16)
        x2c = x2[sl].reshape(NG, 4, 16, 32)          # [g,s,j,v]
        x2t = x2c.transpose(1, 3, 0, 2)              # [s,v,g,j]
        x2bd = np.zeros((128, 65536), NPBF16)
        x2v = x2bd.reshape(4, 32, NG, 64)            # [s,v,g,(s',j)]
        for s in range(4):
            x2v[s, :, :, 16 * s:16 * s + 16] = x2t[s].astype(NPBF16)
        in_maps.append({"x1h": x1h, "wh": wh, "x2bd": x2bd,
                        "a2d": a2d, "bsel": bsel, "c2": c2})
    return in_maps


def unpack_out(outh_all):
    """outh_all: (NCORES*128, 32768) bf16 -> (BATCH, 16, 64) fp32."""
    out = np.empty((BATCH, 16, 64), np.float32)
    for cidx in range(NCORES):
        oc = np.asarray(outh_all[cidx * 128:(cidx + 1) * 128]).astype(np.float32)
        # rows: [tpar(2), H(2), s2(2), c(16)]; cols: [pb(64), gsub(8), o(64)]
        v = oc.reshape(2, 2, 2, 16, 64, 8, 64)
        # b = ((pb*2 + tpar)*8 + gsub)*4 + 2H + s2
        v = v.transpose(4, 0, 5, 1, 2, 3, 6)  # [pb,tpar,gsub,H,s2,c,o]
        out[cidx * B_LOCAL:(cidx + 1) * B_LOCAL] = v.reshape(B_LOCAL, 16, 64)
    return out


class _SpmdRunner:
    """Persistent jitted SPMD executor over the 8 NeuronCores."""

    def __init__(self, nc, n_cores=NCORES):
        install_neuronx_cc_hook()
        self.nc = nc
        self.n_cores = n_cores
        pid_name = nc.partition_id_tensor.name if nc.partition_id_tensor else None

        in_names, out_names, out_avals, zero_outs = [], [], [], []
        for alloc in nc.m.functions[0].allocations:
            if not isinstance(alloc, mybir.MemoryLocationSet):
                continue
            name = alloc.memorylocations[0].name
            if alloc.kind == "ExternalInput":
                if name != pid_name:
                    in_names.append(name)
            elif alloc.kind == "ExternalOutput":
                out_names.append(name)
                shape = tuple(alloc.tensor_shape)
                dtype = mybir.dt.np(alloc.dtype)
                out_avals.append(jax.core.ShapedArray(shape, dtype))
                zero_outs.append(np.zeros(shape, dtype))
        self.in_names, self.out_names = in_names, out_names
        self.out_avals, self.zero_outs = out_avals, zero_outs
        n_params = len(in_names)
        all_names = tuple(in_names + out_names + ([pid_name] if pid_name else []))

        def _body(*args):
            operands = list(args)
            if pid_name is not None:
                operands.append(partition_id_tensor())
            outs = _bass_exec_p.bind(
                *operands,
                out_avals=tuple(out_avals),
                in_names=all_names,
                out_names=tuple(out_names),
                lowering_input_output_aliases=(),
                sim_require_finite=True,
                sim_require_nnan=True,
                nc=nc,
            )
            return tuple(outs)

        devices = jax.devices()[:n_cores]
        self.mesh = Mesh(np.asarray(devices), ("core",))
        self.sharding = NamedSharding(self.mesh, PartitionSpec("core"))
        n_out = len(out_names)
        donate = tuple(range(n_params, n_params + n_out))
        self.jitted = jax.jit(
            shard_map(_body, mesh=self.mesh,
                      in_specs=(PartitionSpec("core"),) * (n_params + n_out),
                      out_specs=(PartitionSpec("core"),) * n_out,
                      check_rep=False),
            donate_argnums=donate, keep_unused=True,
        )

    def stage_inputs(self, in_maps):
        per_core = [[np.asarray(m[name]) for name in self.in_names] for m in in_maps]
        concat = [np.concatenate([per_core[c][i] for c in range(self.n_cores)], axis=0)
                  for i in range(len(self.in_names))]
        return [jax.device_put(a, self.sharding) for a in concat]

    def stage_zeros(self):
        zs = [np.zeros((self.n_cores * z.shape[0], *z.shape[1:]), z.dtype)
              for z in self.zero_outs]
        return [jax.device_put(z, self.sharding) for z in zs]

    def run(self, dev_inputs, dev_zeros=None):
        if dev_zeros is None:
            dev_zeros = self.stage_zeros()
        outs = self.jitted(*dev_inputs, *dev_zeros)
        jax.block_until_ready(outs)
        return outs

    def unshard_out(self, outs):
        i = self.out_names.index("outh")
        return unpack_out(np.asarray(outs[i]))


_RUNNER = None


def _get_runner():
    global _RUNNER
    if _RUNNER is None:
        nc = build_nc()
        _RUNNER = _SpmdRunner(nc, NCORES)
    return _RUNNER


def kernel(x1, x2, w, A, B, C):
    """Full-input entry point. Shards batch across 8 NeuronCores, runs the
    Bass kernel, gathers the full output (32768, 16, 64) float32."""
    runner = _get_runner()
    in_maps = pack_inputs(x1, x2, w, A, B, C)
    dev_in = runner.stage_inputs(in_maps)
    outs = runner.run(dev_in)
    return runner.unshard_out(outs)


# revision 37
# speedup vs baseline: 10.4129x; 1.0098x over previous
"""CP tensor product ('uvu' connection) kernel for Trainium2, SPMD over 8
NeuronCores.

Math per batch element b (decomposed with b = 4g + s, s in 0..3, s = 2H + s2):
  q[b,j,o]  = sum_v x2[b,j,v] w[b,o,v]          (16,64) per-b
  t1[b,r,o] = sum_d A[d,r] x1[b,d,o]            (64,64)
  t3[b,r,o] = sum_j B[j,r] q[b,j,o]             (64,64)
  out[b,c,o]= sum_r C[c,r] t1[b,r,o] t3[b,r,o]  (16,64)

Design (driven by the TimelineSim cost model, where a matmul costs
out_free_cols * 0.42ns at 1 cycle/row for bf16 and contraction size is free):
every matmul packs 128 output partitions by pairing batch elements (s2) with
static block-diagonal stationaries (a2d/bsel/c2), so PE time ~= total output
elements / 128. The per-b q contraction uses per-group block-diagonal x2
stationaries streamed pre-zero-padded from DRAM (LdWeights is free). All
inputs are bf16 (host-converted); PSUM accumulates fp32.

Hardware constraints shaping the pipeline:
  - DVE/Act ops may read at most ONE PSUM operand -> t3 is staged PSUM->SBUF
    (bf16) by the Act engine, then DVE multiplies t1(PSUM) * t3s(SBUF).
  - GPSIMD cannot access PSUM at all -> Act/DVE carry all staging; gpsimd
    only issues the output DMA queue.
  - 8 PSUM banks total: q[1] + t1[2x 1024-col tiles = 4] + t3[2] + out[1].
  - DMA: >=512B contiguous per descriptor, ~1MB per instruction to hide the
    per-instruction ~1.6us overhead; all transfers share one 360B/ns device.

Per-core DRAM layouts (g = group of 4 b's, chunk t = 8 g's, pair = 2 chunks,
super-chunk sc = 8 chunks; local batch 4096):
  x1h  [64,65536]  row 16s+d  (s<2 | s>=2 duplicated rows 32-63.. see pack),
                   col 64g+o : x1[4g+s,d,o]  (rows 0-31 H=0, 32-63 H=1)
  wh   [128,65536] row 32s+v,  col 64g+o : w[4g+s,o,v]
  x2bd [128,65536] row 32s+v,  col 64g+16s'+j : x2[4g+s,j,v] iff s==s', else 0
  a2d  [64,128]    blockdiag2(A) over (s2,d)x(s2,r); rows 32-63 duplicate
  bsel [128,256]   col block H: B[j,r] delta(s,2H+s2); rows 64-127 duplicate
  c2   [128,32]    blockdiag2(C^T) over (s2,r)x(s2,c)
  outh [128,32768] row 32k+16s2+c with k=2*(t%2)+H, col 512*(t//2)+64*(g%8)+o
"""
import os
os.environ.setdefault("JAX_PLATFORMS", "axon,cpu")

import numpy as np
from contextlib import ExitStack

import jax
from jax.experimental.shard_map import shard_map
from jax.sharding import Mesh, PartitionSpec, NamedSharding

import concourse.bass as bass
import concourse.bacc as bacc
import concourse.tile as tile
import concourse.mybir as mybir
from concourse._compat import with_exitstack
from concourse.bass2jax import _bass_exec_p, install_neuronx_cc_hook, partition_id_tensor

F32 = mybir.dt.float32
BF16 = mybir.dt.bfloat16
NPBF16 = mybir.dt.np(BF16)

NCORES = 8
BATCH = 32768
B_LOCAL = BATCH // NCORES   # 4096
NG = B_LOCAL // 4           # 1024 groups of 4 b's
NCHUNK = NG // 8            # 128 chunks (8 groups each)
NPAIR = NCHUNK // 2         # 64 chunk-pairs
NSC = 16                    # super-chunks (8 chunks each)
PAIRS_PER_SC = NPAIR // NSC

# Software-pipeline depths (tuned against TimelineSim):
LAG = 4        # final matmuls trail the t-stage by LAG chunks
QLEAD = 1      # q production leads consumption by QLEAD+1 pairs
SLAB_BUFS = 3  # input half-slab buffers per pool
SC_LEAD = 1    # super-chunk load lead
CONST_Q = 'scalar'   # queue for tiny stationary DMAs (keep sync queue free at start)
X2_DMA_Q = 'sync'    # queue for x2bd half-slab DMAs
TAIL_SPLIT = True    # last super-chunk: per-pair out DMAs
T3SB_BUFS = 4
Q_SPLIT = False


def _emit(ctx: ExitStack, tc: tile.TileContext, outs, ins):
    nc = tc.nc
    (outh,) = outs
    (x1h, wh, x2bd, a2d, bsel, c2) = ins

    const = ctx.enter_context(tc.tile_pool(name="const", bufs=1))
    a2_sb = const.tile([64, 128], BF16)
    bsel_sb = const.tile([128, 256], BF16)
    c2_sb = const.tile([128, 32], BF16)
    cq = getattr(nc, CONST_Q)
    cq.dma_start(a2_sb[:], a2d[:, :])
    cq.dma_start(bsel_sb[:], bsel[:, :])
    cq.dma_start(c2_sb[:], c2[:, :])

    x1_pool = ctx.enter_context(tc.tile_pool(name="x1", bufs=SLAB_BUFS))
    wh_pool = ctx.enter_context(tc.tile_pool(name="wh", bufs=SLAB_BUFS))
    x2_pool = ctx.enter_context(tc.tile_pool(name="x2", bufs=SLAB_BUFS))
    qsb_pool = ctx.enter_context(tc.tile_pool(name="qsb", bufs=max(2, QLEAD + 1)))
    msb_pool = ctx.enter_context(tc.tile_pool(name="msb", bufs=LAG + 1))
    t3sb_pool = ctx.enter_context(tc.tile_pool(name="t3sb", bufs=T3SB_BUFS))
    osb_pool = ctx.enter_context(tc.tile_pool(name="osb", bufs=2))
    pq = ctx.enter_context(tc.tile_pool(name="pq", bufs=1, space="PSUM"))
    pt1 = ctx.enter_context(tc.tile_pool(name="pt1", bufs=2, space="PSUM"))
    pt3 = ctx.enter_context(tc.tile_pool(name="pt3", bufs=1, space="PSUM"))
    po = ctx.enter_context(tc.tile_pool(name="po", bufs=1, space="PSUM"))

    slabs = {}   # sc -> (x1_halves, w_halves, x2_halves, q_tile)
    m_of = {}    # chunk -> m_sb tile [128,1024] (cols 0:512 H=0, 512: H=1)
    o_of = {}    # pair -> out psum tile
    osb_of = {}  # sc -> out sbuf slab

    def load_sc(sc):
        # two half-slabs per tensor so compute can start after half a load
        x1_h, w_h, x2_h = [], [], []
        for h in range(2):
            cw = slice(4096 * sc + 2048 * h, 4096 * sc + 2048 * (h + 1))
            x1_t = x1_pool.tile([64, 2048], BF16, name="x1s")
            w_t = wh_pool.tile([128, 2048], BF16, name="ws")
            x2_t = x2_pool.tile([128, 2048], BF16, name="x2s")
            if sc == 0:
                # q consumes x2/w first; x1 is needed only once t1 starts
                getattr(nc, X2_DMA_Q).dma_start(x2_t[:], x2bd[:, cw])
                nc.sync.dma_start(w_t[:], wh[:, cw])
                nc.sync.dma_start(x1_t[:], x1h[:, cw])
            else:
                nc.sync.dma_start(x1_t[:], x1h[:, cw])
                nc.sync.dma_start(w_t[:], wh[:, cw])
                getattr(nc, X2_DMA_Q).dma_start(x2_t[:], x2bd[:, cw])
            x1_h.append(x1_t); w_h.append(w_t); x2_h.append(x2_t)
        q_t = qsb_pool.tile([128, 2048], BF16, name="qs")
        slabs[sc] = (x1_h, w_h, x2_h, q_t)

    def emit_q(pair):
        # 16 q matmuls (one per group) for both chunks of `pair` into one
        # [128,512] psum bank (chunk parity -> partition offset 0/64), then
        # one Act copy stages it to SBUF bf16.
        (x1_h, w_h, x2_h, q_t) = slabs[pair // PAIRS_PER_SC]
        ps = pq.tile([128, 512], F32, name="qps")
        for tp in range(2):            # chunk t = 2*pair + tp
            t = 2 * pair + tp
            for gg in range(8):        # group g = 8*t + gg
                gcol = 64 * ((t % 8) * 8 + gg)   # col offset within slab
                nc.tensor.matmul(
                    ps[64 * tp:64 * tp + 64, 64 * gg:64 * gg + 64],
                    x2_h[gcol // 2048][:, gcol % 2048:gcol % 2048 + 64],
                    w_h[gcol // 2048][:, gcol % 2048:gcol % 2048 + 64],
                    tile_position=(0, 64 * tp),
                )
        pb = pair % PAIRS_PER_SC
        if Q_SPLIT:
            nc.scalar.copy(q_t[:, 512 * pb:512 * pb + 256], ps[:, 0:256])
            nc.vector.tensor_scalar_mul(
                q_t[:, 512 * pb + 256:512 * (pb + 1)], ps[:, 256:512], 1.0)
        else:
            nc.scalar.copy(q_t[:, 512 * pb:512 * (pb + 1)], ps[:])

    def emit_tstage(t):
        # t1/t3 matmuls (H-interleaved), Act stages t3 to SBUF bf16, DVE
        # multiplies m = t1 * t3s.
        (x1_h, w_h, x2_h, q_t) = slabs[t // 8]
        x1_t = x1_h[(t % 8) // 4]
        tp = t % 2
        cw0 = 512 * ((t % 8) % 4)
        colw = slice(cw0, cw0 + 512)
        qcw = slice(512 * ((t % 8) // 2), 512 * ((t % 8) // 2) + 512)
        t1 = pt1.tile([128, 1024], F32, name="t1ps")
        t3 = pt3.tile([128, 1024], F32, name="t3ps")
        for H in range(2):
            nc.tensor.matmul(
                t1[:, 512 * H:512 * H + 512],
                a2_sb[32 * H:32 * H + 32, :],
                x1_t[32 * H:32 * H + 32, colw],
                tile_position=(32 * H, 0),
            )
            nc.tensor.matmul(
                t3[:, 512 * H:512 * H + 512],
                bsel_sb[64 * tp:64 * tp + 64, 128 * H:128 * (H + 1)],
                q_t[64 * tp:64 * tp + 64, qcw],
                tile_position=(64 * tp, 0),
            )
        t3s = t3sb_pool.tile([128, 1024], BF16, name="t3s")
        nc.scalar.copy(t3s[:], t3[:])
        m = msb_pool.tile([128, 1024], BF16, name="ms")
        nc.vector.tensor_mul(m[:], t1[:], t3s[:])
        m_of[t] = m

    def emit_finals(t):
        # C contraction for chunk t into the pair's out psum bank (4 stacked
        # 32-partition strips), staged out and DMAed per super-chunk.
        pair = t // 2
        tp = t % 2
        if tp == 0:
            o_of[pair] = po.tile([128, 512], F32, name="ops")
        o_ps = o_of[pair]
        m = m_of.pop(t)
        for H in range(2):
            k = 2 * tp + H
            nc.tensor.matmul(
                o_ps[32 * k:32 * k + 32, :], c2_sb[:],
                m[:, 512 * H:512 * H + 512],
                tile_position=(0, 32 * k),
            )
        if tp == 1:
            sc = pair // PAIRS_PER_SC
            if sc not in osb_of:
                osb_of[sc] = osb_pool.tile([128, 2048], BF16, name="osb")
            pp = pair % PAIRS_PER_SC
            odst = osb_of[sc][:, 512 * pp:512 * (pp + 1)]
            ops = o_of.pop(pair)
            if pair % 2 == 0:
                nc.vector.tensor_scalar_mul(odst, ops[:], 1.0)
            else:
                nc.scalar.copy(odst, ops[:])
            if TAIL_SPLIT and sc == NSC - 1:
                nc.sync.dma_start(
                    outh[:, 2048 * sc + 512 * pp:2048 * sc + 512 * (pp + 1)],
                    osb_of[sc][:, 512 * pp:512 * (pp + 1)])
                if pp == PAIRS_PER_SC - 1:
                    osb_of.pop(sc)
            elif pp == PAIRS_PER_SC - 1:
                nc.sync.dma_start(outh[:, 2048 * sc:2048 * (sc + 1)],
                                  osb_of.pop(sc)[:])

    for k in range(SC_LEAD + 1):
        load_sc(k)
    for p in range(QLEAD + 1):
        emit_q(p)
    for t in range(NCHUNK + LAG):
        if t < NCHUNK:
            if t % 8 == 0 and t // 8 + SC_LEAD + 1 < NSC:
                load_sc(t // 8 + SC_LEAD + 1)
            if t % 2 == 0 and t // 2 + QLEAD + 1 < NPAIR:
                emit_q(t // 2 + QLEAD + 1)
            emit_tstage(t)
            if t >= LAG:
                emit_finals(t - LAG)
        else:
            emit_finals(t - LAG)


@with_exitstack
def _cp_kernel(ctx, tc, outs, ins):
    _emit(ctx, tc, outs, ins)


def build_nc():
    nc = bacc.Bacc("TRN2", target_bir_lowering=False, debug=False)
    x1h = nc.dram_tensor("x1h", [64, 65536], BF16, kind="ExternalInput").ap()
    wh = nc.dram_tensor("wh", [128, 65536], BF16, kind="ExternalInput").ap()
    x2bd = nc.dram_tensor("x2bd", [128, 65536], BF16, kind="ExternalInput").ap()
    a2d = nc.dram_tensor("a2d", [64, 128], BF16, kind="ExternalInput").ap()
    bsel = nc.dram_tensor("bsel", [128, 256], BF16, kind="ExternalInput").ap()
    c2 = nc.dram_tensor("c2", [128, 32], BF16, kind="ExternalInput").ap()
    outh = nc.dram_tensor("outh", [128, 32768], BF16, kind="ExternalOutput").ap()
    with tile.TileContext(nc, trace_sim=False) as tc:
        _cp_kernel(tc, [outh], [x1h, wh, x2bd, a2d, bsel, c2])
    nc.compile()
    return nc


def pack_inputs(x1, x2, w, A, B, C):
    """Host-side: full fp32 arrays -> per-core bf16 packed arrays (list of
    dicts keyed by dram tensor name)."""
    x1 = np.asarray(x1, np.float32)
    x2 = np.asarray(x2, np.float32)
    w = np.asarray(w, np.float32)
    A = np.asarray(A, np.float32)
    B = np.asarray(B, np.float32)
    C = np.asarray(C, np.float32)

    a2d = np.zeros((64, 128), np.float32)
    for s2 in range(2):
        a2d[16 * s2:16 * s2 + 16, 64 * s2:64 * s2 + 64] = A
    a2d[32:64] = a2d[0:32]
    bsel = np.zeros((128, 256), np.float32)
    for H in range(2):
        for s2 in range(2):
            s = 2 * H + s2
            bsel[16 * s:16 * s + 16, 128 * H + 64 * s2:128 * H + 64 * s2 + 64] = B
    bsel[64:128] = bsel[0:64]
    c2 = np.zeros((128, 32), np.float32)
    for s2 in range(2):
        c2[64 * s2:64 * s2 + 64, 16 * s2:16 * s2 + 16] = C.T

    a2d = a2d.astype(NPBF16)
    bsel = bsel.astype(NPBF16)
    c2 = c2.astype(NPBF16)

    in_maps = []
    for cidx in range(NCORES):
        sl = slice(cidx * B_LOCAL, (cidx + 1) * B_LOCAL)
        x1c = x1[sl].reshape(NG, 4, 16, 64)          # [g,s,d,o]
        x1h = np.ascontiguousarray(
            x1c.transpose(1, 2, 0, 3)).reshape(64, 65536).astype(NPBF16)
        wc = w[sl].reshape(NG, 4, 64, 32)            # [g,s,o,v]
        wh = np.ascontiguousarray(
            wc.transpose(1, 3, 0, 2)).reshape(128, 65536).astype(NPBF16)
        x2c = x2[sl].reshape(NG, 4, 16, 32)          # [g,s,j,v]
        x2t = x2c.transpose(1, 3, 0, 2)              # [s,v,g,j]
        x2bd = np.zeros((128, 65536), NPBF16)
        x2v = x2bd.reshape(4, 32, NG, 64)            # [s,v,g,(s',j)]
        for s in range(4):
            x2v[s, :, :, 16 * s:16 * s + 16] = x2t[s].astype(NPBF16)
        in_maps.append({"x1h": x1h, "wh": wh, "x2bd": x2bd,
                        "a2d": a2d, "bsel": bsel, "c2": c2})
    return in_maps


def unpack_out(outh_all):
    """outh_all: (NCORES*128, 32768) bf16 -> (BATCH, 16, 64) fp32."""
    out = np.empty((BATCH, 16, 64), np.float32)
    for cidx in range(NCORES):
        oc = np.asarray(outh_all[cidx * 128:(cidx + 1) * 128]).astype(np.float32)
        # rows: [tpar(2), H(2), s2(2), c(16)]; cols: [pb(64), gsub(8), o(64)]
        v = oc.reshape(2, 2, 2, 16, 64, 8, 64)
        # b = ((pb*2 + tpar)*8 + gsub)*4 + 2H + s2
        v = v.transpose(4, 0, 5, 1, 2, 3, 6)  # [pb,tpar,gsub,H,s2,c,o]
        out[cidx * B_LOCAL:(cidx + 1) * B_LOCAL] = v.reshape(B_LOCAL, 16, 64)
    return out


class _SpmdRunner:
    """Persistent jitted SPMD executor over the 8 NeuronCores."""

    def __init__(self, nc, n_cores=NCORES):
        install_neuronx_cc_hook()
        self.nc = nc
        self.n_cores = n_cores
        pid_name = nc.partition_id_tensor.name if nc.partition_id_tensor else None

        in_names, out_names, out_avals, zero_outs = [], [], [], []
        for alloc in nc.m.functions[0].allocations:
            if not isinstance(alloc, mybir.MemoryLocationSet):
                continue
            name = alloc.memorylocations[0].name
            if alloc.kind == "ExternalInput":
                if name != pid_name:
                    in_names.append(name)
            elif alloc.kind == "ExternalOutput":
                out_names.append(name)
                shape = tuple(alloc.tensor_shape)
                dtype = mybir.dt.np(alloc.dtype)
                out_avals.append(jax.core.ShapedArray(shape, dtype))
                zero_outs.append(np.zeros(shape, dtype))
        self.in_names, self.out_names = in_names, out_names
        self.out_avals, self.zero_outs = out_avals, zero_outs
        n_params = len(in_names)
        all_names = tuple(in_names + out_names + ([pid_name] if pid_name else []))

        def _body(*args):
            operands = list(args)
            if pid_name is not None:
                operands.append(partition_id_tensor())
            outs = _bass_exec_p.bind(
                *operands,
                out_avals=tuple(out_avals),
                in_names=all_names,
                out_names=tuple(out_names),
                lowering_input_output_aliases=(),
                sim_require_finite=True,
                sim_require_nnan=True,
                nc=nc,
            )
            return tuple(outs)

        devices = jax.devices()[:n_cores]
        self.mesh = Mesh(np.asarray(devices), ("core",))
        self.sharding = NamedSharding(self.mesh, PartitionSpec("core"))
        n_out = len(out_names)
        donate = tuple(range(n_params, n_params + n_out))
        self.jitted = jax.jit(
            shard_map(_body, mesh=self.mesh,
                      in_specs=(PartitionSpec("core"),) * (n_params + n_out),
                      out_specs=(PartitionSpec("core"),) * n_out,
                      check_rep=False),
            donate_argnums=donate, keep_unused=True,
        )

    def stage_inputs(self, in_maps):
        per_core = [[np.asarray(m[name]) for name in self.in_names] for m in in_maps]
        concat = [np.concatenate([per_core[c][i] for c in range(self.n_cores)], axis=0)
                  for i in range(len(self.in_names))]
        return [jax.device_put(a, self.sharding) for a in concat]

    def stage_zeros(self):
        zs = [np.zeros((self.n_cores * z.shape[0], *z.shape[1:]), z.dtype)
              for z in self.zero_outs]
        return [jax.device_put(z, self.sharding) for z in zs]

    def run(self, dev_inputs, dev_zeros=None):
        if dev_zeros is None:
            dev_zeros = self.stage_zeros()
        outs = self.jitted(*dev_inputs, *dev_zeros)
        jax.block_until_ready(outs)
        return outs

    def unshard_out(self, outs):
        i = self.out_names.index("outh")
        return unpack_out(np.asarray(outs[i]))


_RUNNER = None


def _get_runner():
    global _RUNNER
    if _RUNNER is None:
        nc = build_nc()
        _RUNNER = _SpmdRunner(nc, NCORES)
    return _RUNNER


def kernel(x1, x2, w, A, B, C):
    """Full-input entry point. Shards batch across 8 NeuronCores, runs the
    Bass kernel, gathers the full output (32768, 16, 64) float32."""
    runner = _get_runner()
    in_maps = pack_inputs(x1, x2, w, A, B, C)
    dev_in = runner.stage_inputs(in_maps)
    outs = runner.run(dev_in)
    return runner.unshard_out(outs)


# revision 41
# speedup vs baseline: 10.4421x; 1.0028x over previous
"""CP tensor product ('uvu' connection) kernel for Trainium2, SPMD over 8
NeuronCores.

Math per batch element b (decomposed with b = 4g + s, s in 0..3, s = 2H + s2):
  q[b,j,o]  = sum_v x2[b,j,v] w[b,o,v]          (16,64) per-b
  t1[b,r,o] = sum_d A[d,r] x1[b,d,o]            (64,64)
  t3[b,r,o] = sum_j B[j,r] q[b,j,o]             (64,64)
  out[b,c,o]= sum_r C[c,r] t1[b,r,o] t3[b,r,o]  (16,64)

Design (driven by the TimelineSim cost model, where a matmul costs
out_free_cols * 0.42ns at 1 cycle/row for bf16 and contraction size is free):
every matmul packs 128 output partitions by pairing batch elements (s2) with
static block-diagonal stationaries (a2d/bsel/c2), so PE time ~= total output
elements / 128. The per-b q contraction uses per-group block-diagonal x2
stationaries streamed pre-zero-padded from DRAM (LdWeights is free). All
inputs are bf16 (host-converted); PSUM accumulates fp32.

Hardware constraints shaping the pipeline:
  - DVE/Act ops may read at most ONE PSUM operand -> t3 is staged PSUM->SBUF
    (bf16) by the Act engine, then DVE multiplies t1(PSUM) * t3s(SBUF).
  - GPSIMD cannot access PSUM at all -> Act/DVE carry all staging; gpsimd
    only issues the output DMA queue.
  - 8 PSUM banks total: q[1] + t1[2x 1024-col tiles = 4] + t3[2] + out[1].
  - DMA: >=512B contiguous per descriptor, ~1MB per instruction to hide the
    per-instruction ~1.6us overhead; all transfers share one 360B/ns device.

Per-core DRAM layouts (g = group of 4 b's, chunk t = 8 g's, pair = 2 chunks,
super-chunk sc = 8 chunks; local batch 4096):
  x1h  [64,65536]  row 16s+d  (s<2 | s>=2 duplicated rows 32-63.. see pack),
                   col 64g+o : x1[4g+s,d,o]  (rows 0-31 H=0, 32-63 H=1)
  wh   [128,65536] row 32s+v,  col 64g+o : w[4g+s,o,v]
  x2bd [128,65536] row 32s+v,  col 64g+16s'+j : x2[4g+s,j,v] iff s==s', else 0
  a2d  [64,128]    blockdiag2(A) over (s2,d)x(s2,r); rows 32-63 duplicate
  bsel [128,256]   col block H: B[j,r] delta(s,2H+s2); rows 64-127 duplicate
  c2   [128,32]    blockdiag2(C^T) over (s2,r)x(s2,c)
  outh [128,32768] row 32k+16s2+c with k=2*(t%2)+H, col 512*(t//2)+64*(g%8)+o
"""
import os
os.environ.setdefault("JAX_PLATFORMS", "axon,cpu")

import numpy as np
from contextlib import ExitStack

import jax
from jax.experimental.shard_map import shard_map
from jax.sharding import Mesh, PartitionSpec, NamedSharding

import concourse.bass as bass
import concourse.bacc as bacc
import concourse.tile as tile
import concourse.mybir as mybir
from concourse._compat import with_exitstack
from concourse.bass2jax import _bass_exec_p, install_neuronx_cc_hook, partition_id_tensor

F32 = mybir.dt.float32
BF16 = mybir.dt.bfloat16
NPBF16 = mybir.dt.np(BF16)

NCORES = 8
BATCH = 32768
B_LOCAL = BATCH // NCORES   # 4096
NG = B_LOCAL // 4           # 1024 groups of 4 b's
NCHUNK = NG // 8            # 128 chunks (8 groups each)
NPAIR = NCHUNK // 2         # 64 chunk-pairs
NSC = 16                    # super-chunks (8 chunks each)
PAIRS_PER_SC = NPAIR // NSC

# Software-pipeline depths (tuned against TimelineSim):
LAG = 4        # final matmuls trail the t-stage by LAG chunks
QLEAD = 1      # q production leads consumption by QLEAD+1 pairs
SLAB_BUFS = 3  # input half-slab buffers per pool
SC_LEAD = 1    # super-chunk load lead
CONST_Q = 'scalar'   # queue for tiny stationary DMAs (keep sync queue free at start)
X2_DMA_Q = 'sync'    # queue for x2bd half-slab DMAs
TAIL_SPLIT = True    # last super-chunk: per-pair out DMAs
T3SB_BUFS = 4
Q_SPLIT = False
NWARM = 8   # dummy PE matmuls to burn through the p-state ramp during initial loads


def _emit(ctx: ExitStack, tc: tile.TileContext, outs, ins):
    nc = tc.nc
    (outh,) = outs
    (x1h, wh, x2bd, a2d, bsel, c2) = ins

    const = ctx.enter_context(tc.tile_pool(name="const", bufs=1))
    a2_sb = const.tile([64, 128], BF16)
    bsel_sb = const.tile([128, 256], BF16)
    c2_sb = const.tile([128, 32], BF16)
    cq = getattr(nc, CONST_Q)
    cq.dma_start(a2_sb[:], a2d[:, :])
    cq.dma_start(bsel_sb[:], bsel[:, :])
    cq.dma_start(c2_sb[:], c2[:, :])

    x1_pool = ctx.enter_context(tc.tile_pool(name="x1", bufs=SLAB_BUFS))
    wh_pool = ctx.enter_context(tc.tile_pool(name="wh", bufs=SLAB_BUFS))
    x2_pool = ctx.enter_context(tc.tile_pool(name="x2", bufs=SLAB_BUFS))
    qsb_pool = ctx.enter_context(tc.tile_pool(name="qsb", bufs=max(2, QLEAD + 1)))
    msb_pool = ctx.enter_context(tc.tile_pool(name="msb", bufs=LAG + 1))
    t3sb_pool = ctx.enter_context(tc.tile_pool(name="t3sb", bufs=T3SB_BUFS))
    osb_pool = ctx.enter_context(tc.tile_pool(name="osb", bufs=2))
    pq = ctx.enter_context(tc.tile_pool(name="pq", bufs=1, space="PSUM"))
    pt1 = ctx.enter_context(tc.tile_pool(name="pt1", bufs=2, space="PSUM"))
    pt3 = ctx.enter_context(tc.tile_pool(name="pt3", bufs=1, space="PSUM"))
    po = ctx.enter_context(tc.tile_pool(name="po", bufs=1, space="PSUM"))

    if NWARM:
        # PE p-state warmup: harmless matmuls while the first slabs stream in
        wlhs = const.tile([1, 1], BF16)
        wrhs = const.tile([1, 512], BF16)
        nc.vector.memset(wlhs[:], 0.0)
        nc.vector.memset(wrhs[:], 0.0)
        warm_state = {"lhs": wlhs, "rhs": wrhs}
    else:
        warm_state = None

    slabs = {}   # sc -> (x1_halves, w_halves, x2_halves, q_tile)
    m_of = {}    # chunk -> m_sb tile [128,1024] (cols 0:512 H=0, 512: H=1)
    o_of = {}    # pair -> out psum tile
    osb_of = {}  # sc -> out sbuf slab

    def load_sc(sc):
        # two half-slabs per tensor so compute can start after half a load
        x1_h, w_h, x2_h = [], [], []
        for h in range(2):
            cw = slice(4096 * sc + 2048 * h, 4096 * sc + 2048 * (h + 1))
            x1_t = x1_pool.tile([64, 2048], BF16, name="x1s")
            w_t = wh_pool.tile([128, 2048], BF16, name="ws")
            x2_t = x2_pool.tile([128, 2048], BF16, name="x2s")
            if sc == 0:
                # q consumes x2/w first; x1 is needed only once t1 starts
                getattr(nc, X2_DMA_Q).dma_start(x2_t[:], x2bd[:, cw])
                nc.sync.dma_start(w_t[:], wh[:, cw])
                nc.sync.dma_start(x1_t[:], x1h[:, cw])
            else:
                nc.sync.dma_start(x1_t[:], x1h[:, cw])
                nc.sync.dma_start(w_t[:], wh[:, cw])
                getattr(nc, X2_DMA_Q).dma_start(x2_t[:], x2bd[:, cw])
            x1_h.append(x1_t); w_h.append(w_t); x2_h.append(x2_t)
        q_t = qsb_pool.tile([128, 2048], BF16, name="qs")
        slabs[sc] = (x1_h, w_h, x2_h, q_t)

    def emit_q(pair):
        # 16 q matmuls (one per group) for both chunks of `pair` into one
        # [128,512] psum bank (chunk parity -> partition offset 0/64), then
        # one Act copy stages it to SBUF bf16.
        (x1_h, w_h, x2_h, q_t) = slabs[pair // PAIRS_PER_SC]
        ps = pq.tile([128, 512], F32, name="qps")
        if pair == 0 and warm_state:
            # PE p-state warmup into q(0)'s own bank while slabs stream in;
            # the real q matmuls below reset their regions (start=True).
            for _ in range(NWARM):
                nc.tensor.matmul(ps[0:1, :], warm_state["lhs"][:],
                                 warm_state["rhs"][:])
        for tp in range(2):            # chunk t = 2*pair + tp
            t = 2 * pair + tp
            for gg in range(8):        # group g = 8*t + gg
                gcol = 64 * ((t % 8) * 8 + gg)   # col offset within slab
                nc.tensor.matmul(
                    ps[64 * tp:64 * tp + 64, 64 * gg:64 * gg + 64],
                    x2_h[gcol // 2048][:, gcol % 2048:gcol % 2048 + 64],
                    w_h[gcol // 2048][:, gcol % 2048:gcol % 2048 + 64],
                    tile_position=(0, 64 * tp),
                )
        pb = pair % PAIRS_PER_SC
        if Q_SPLIT:
            nc.scalar.copy(q_t[:, 512 * pb:512 * pb + 256], ps[:, 0:256])
            nc.vector.tensor_scalar_mul(
                q_t[:, 512 * pb + 256:512 * (pb + 1)], ps[:, 256:512], 1.0)
        else:
            nc.scalar.copy(q_t[:, 512 * pb:512 * (pb + 1)], ps[:])

    def emit_tstage(t):
        # t1/t3 matmuls (H-interleaved), Act stages t3 to SBUF bf16, DVE
        # multiplies m = t1 * t3s.
        (x1_h, w_h, x2_h, q_t) = slabs[t // 8]
        x1_t = x1_h[(t % 8) // 4]
        tp = t % 2
        cw0 = 512 * ((t % 8) % 4)
        colw = slice(cw0, cw0 + 512)
        qcw = slice(512 * ((t % 8) // 2), 512 * ((t % 8) // 2) + 512)
        t1 = pt1.tile([128, 1024], F32, name="t1ps")
        t3 = pt3.tile([128, 1024], F32, name="t3ps")
        for H in range(2):
            nc.tensor.matmul(
                t1[:, 512 * H:512 * H + 512],
                a2_sb[32 * H:32 * H + 32, :],
                x1_t[32 * H:32 * H + 32, colw],
                tile_position=(32 * H, 0),
            )
            nc.tensor.matmul(
                t3[:, 512 * H:512 * H + 512],
                bsel_sb[64 * tp:64 * tp + 64, 128 * H:128 * (H + 1)],
                q_t[64 * tp:64 * tp + 64, qcw],
                tile_position=(64 * tp, 0),
            )
        t3s = t3sb_pool.tile([128, 1024], BF16, name="t3s")
        nc.scalar.copy(t3s[:], t3[:])
        m = msb_pool.tile([128, 1024], BF16, name="ms")
        nc.vector.tensor_mul(m[:], t1[:], t3s[:])
        m_of[t] = m

    def emit_finals(t):
        # C contraction for chunk t into the pair's out psum bank (4 stacked
        # 32-partition strips), staged out and DMAed per super-chunk.
        pair = t // 2
        tp = t % 2
        if tp == 0:
            o_of[pair] = po.tile([128, 512], F32, name="ops")
        o_ps = o_of[pair]
        m = m_of.pop(t)
        for H in range(2):
            k = 2 * tp + H
            nc.tensor.matmul(
                o_ps[32 * k:32 * k + 32, :], c2_sb[:],
                m[:, 512 * H:512 * H + 512],
                tile_position=(0, 32 * k),
            )
        if tp == 1:
            sc = pair // PAIRS_PER_SC
            if sc not in osb_of:
                osb_of[sc] = osb_pool.tile([128, 2048], BF16, name="osb")
            pp = pair % PAIRS_PER_SC
            odst = osb_of[sc][:, 512 * pp:512 * (pp + 1)]
            ops = o_of.pop(pair)
            if pair % 2 == 0:
                nc.vector.tensor_scalar_mul(odst, ops[:], 1.0)
            else:
                nc.scalar.copy(odst, ops[:])
            if TAIL_SPLIT and sc == NSC - 1:
                nc.sync.dma_start(
                    outh[:, 2048 * sc + 512 * pp:2048 * sc + 512 * (pp + 1)],
                    osb_of[sc][:, 512 * pp:512 * (pp + 1)])
                if pp == PAIRS_PER_SC - 1:
                    osb_of.pop(sc)
            elif pp == PAIRS_PER_SC - 1:
                nc.sync.dma_start(outh[:, 2048 * sc:2048 * (sc + 1)],
                                  osb_of.pop(sc)[:])

    for k in range(SC_LEAD + 1):
        load_sc(k)
    for p in range(QLEAD + 1):
        emit_q(p)
    for t in range(NCHUNK + LAG):
        if t < NCHUNK:
            if t % 8 == 0 and t // 8 + SC_LEAD + 1 < NSC:
                load_sc(t // 8 + SC_LEAD + 1)
            if t % 2 == 0 and t // 2 + QLEAD + 1 < NPAIR:
                emit_q(t // 2 + QLEAD + 1)
            emit_tstage(t)
            if t >= LAG:
                emit_finals(t - LAG)
        else:
            emit_finals(t - LAG)


@with_exitstack
def _cp_kernel(ctx, tc, outs, ins):
    _emit(ctx, tc, outs, ins)


def build_nc():
    nc = bacc.Bacc("TRN2", target_bir_lowering=False, debug=False)
    x1h = nc.dram_tensor("x1h", [64, 65536], BF16, kind="ExternalInput").ap()
    wh = nc.dram_tensor("wh", [128, 65536], BF16, kind="ExternalInput").ap()
    x2bd = nc.dram_tensor("x2bd", [128, 65536], BF16, kind="ExternalInput").ap()
    a2d = nc.dram_tensor("a2d", [64, 128], BF16, kind="ExternalInput").ap()
    bsel = nc.dram_tensor("bsel", [128, 256], BF16, kind="ExternalInput").ap()
    c2 = nc.dram_tensor("c2", [128, 32], BF16, kind="ExternalInput").ap()
    outh = nc.dram_tensor("outh", [128, 32768], BF16, kind="ExternalOutput").ap()
    with tile.TileContext(nc, trace_sim=False) as tc:
        _cp_kernel(tc, [outh], [x1h, wh, x2bd, a2d, bsel, c2])
    nc.compile()
    return nc


def pack_inputs(x1, x2, w, A, B, C):
    """Host-side: full fp32 arrays -> per-core bf16 packed arrays (list of
    dicts keyed by dram tensor name)."""
    x1 = np.asarray(x1, np.float32)
    x2 = np.asarray(x2, np.float32)
    w = np.asarray(w, np.float32)
    A = np.asarray(A, np.float32)
    B = np.asarray(B, np.float32)
    C = np.asarray(C, np.float32)

    a2d = np.zeros((64, 128), np.float32)
    for s2 in range(2):
        a2d[16 * s2:16 * s2 + 16, 64 * s2:64 * s2 + 64] = A
    a2d[32:64] = a2d[0:32]
    bsel = np.zeros((128, 256), np.float32)
    for H in range(2):
        for s2 in range(2):
            s = 2 * H + s2
            bsel[16 * s:16 * s + 16, 128 * H + 64 * s2:128 * H + 64 * s2 + 64] = B
    bsel[64:128] = bsel[0:64]
    c2 = np.zeros((128, 32), np.float32)
    for s2 in range(2):
        c2[64 * s2:64 * s2 + 64, 16 * s2:16 * s2 + 16] = C.T

    a2d = a2d.astype(NPBF16)
    bsel = bsel.astype(NPBF16)
    c2 = c2.astype(NPBF16)

    in_maps = []
    for cidx in range(NCORES):
        sl = slice(cidx * B_LOCAL, (cidx + 1) * B_LOCAL)
        x1c = x1[sl].reshape(NG, 4, 16, 64)          # [g,s,d,o]
        x1h = np.ascontiguousarray(
            x1c.transpose(1, 2, 0, 3)).reshape(64, 65536).astype(NPBF16)
        wc = w[sl].reshape(NG, 4, 64, 32)            # [g,s,o,v]
        wh = np.ascontiguousarray(
            wc.transpose(1, 3, 0, 2)).reshape(128, 65536).astype(NPBF16)
        x2c = x2[sl].reshape(NG, 4, 16, 32)          # [g,s,j,v]
        x2t = x2c.transpose(1, 3, 0, 2)              # [s,v,g,j]
        x2bd = np.zeros((128, 65536), NPBF16)
        x2v = x2bd.reshape(4, 32, NG, 64)            # [s,v,g,(s',j)]
        for s in range(4):
            x2v[s, :, :, 16 * s:16 * s + 16] = x2t[s].astype(NPBF16)
        in_maps.append({"x1h": x1h, "wh": wh, "x2bd": x2bd,
                        "a2d": a2d, "bsel": bsel, "c2": c2})
    return in_maps


def unpack_out(outh_all):
    """outh_all: (NCORES*128, 32768) bf16 -> (BATCH, 16, 64) fp32."""
    out = np.empty((BATCH, 16, 64), np.float32)
    for cidx in range(NCORES):
        oc = np.asarray(outh_all[cidx * 128:(cidx + 1) * 128]).astype(np.float32)
        # rows: [tpar(2), H(2), s2(2), c(16)]; cols: [pb(64), gsub(8), o(64)]
        v = oc.reshape(2, 2, 2, 16, 64, 8, 64)
        # b = ((pb*2 + tpar)*8 + gsub)*4 + 2H + s2
        v = v.transpose(4, 0, 5, 1, 2, 3, 6)  # [pb,tpar,gsub,H,s2,c,o]
        out[cidx * B_LOCAL:(cidx + 1) * B_LOCAL] = v.reshape(B_LOCAL, 16, 64)
    return out


class _SpmdRunner:
    """Persistent jitted SPMD executor over the 8 NeuronCores."""

    def __init__(self, nc, n_cores=NCORES):
        install_neuronx_cc_hook()
        self.nc = nc
        self.n_cores = n_cores
        pid_name = nc.partition_id_tensor.name if nc.partition_id_tensor else None

        in_names, out_names, out_avals, zero_outs = [], [], [], []
        for alloc in nc.m.functions[0].allocations:
            if not isinstance(alloc, mybir.MemoryLocationSet):
                continue
            name = alloc.memorylocations[0].name
            if alloc.kind == "ExternalInput":
                if name != pid_name:
                    in_names.append(name)
            elif alloc.kind == "ExternalOutput":
                out_names.append(name)
                shape = tuple(alloc.tensor_shape)
                dtype = mybir.dt.np(alloc.dtype)
                out_avals.append(jax.core.ShapedArray(shape, dtype))
                zero_outs.append(np.zeros(shape, dtype))
        self.in_names, self.out_names = in_names, out_names
        self.out_avals, self.zero_outs = out_avals, zero_outs
        n_params = len(in_names)
        all_names = tuple(in_names + out_names + ([pid_name] if pid_name else []))

        def _body(*args):
            operands = list(args)
            if pid_name is not None:
                operands.append(partition_id_tensor())
            outs = _bass_exec_p.bind(
                *operands,
                out_avals=tuple(out_avals),
                in_names=all_names,
                out_names=tuple(out_names),
                lowering_input_output_aliases=(),
                sim_require_finite=True,
                sim_require_nnan=True,
                nc=nc,
            )
            return tuple(outs)

        devices = jax.devices()[:n_cores]
        self.mesh = Mesh(np.asarray(devices), ("core",))
        self.sharding = NamedSharding(self.mesh, PartitionSpec("core"))
        n_out = len(out_names)
        donate = tuple(range(n_params, n_params + n_out))
        self.jitted = jax.jit(
            shard_map(_body, mesh=self.mesh,
                      in_specs=(PartitionSpec("core"),) * (n_params + n_out),
                      out_specs=(PartitionSpec("core"),) * n_out,
                      check_rep=False),
            donate_argnums=donate, keep_unused=True,
        )

    def stage_inputs(self, in_maps):
        per_core = [[np.asarray(m[name]) for name in self.in_names] for m in in_maps]
        concat = [np.concatenate([per_core[c][i] for c in range(self.n_cores)], axis=0)
                  for i in range(len(self.in_names))]
        return [jax.device_put(a, self.sharding) for a in concat]

    def stage_zeros(self):
        zs = [np.zeros((self.n_cores * z.shape[0], *z.shape[1:]), z.dtype)
              for z in self.zero_outs]
        return [jax.device_put(z, self.sharding) for z in zs]

    def run(self, dev_inputs, dev_zeros=None):
        if dev_zeros is None:
            dev_zeros = self.stage_zeros()
        outs = self.jitted(*dev_inputs, *dev_zeros)
        jax.block_until_ready(outs)
        return outs

    def unshard_out(self, outs):
        i = self.out_names.index("outh")
        return unpack_out(np.asarray(outs[i]))


_RUNNER = None


def _get_runner():
    global _RUNNER
    if _RUNNER is None:
        nc = build_nc()
        _RUNNER = _SpmdRunner(nc, NCORES)
    return _RUNNER


def kernel(x1, x2, w, A, B, C):
    """Full-input entry point. Shards batch across 8 NeuronCores, runs the
    Bass kernel, gathers the full output (32768, 16, 64) float32."""
    runner = _get_runner()
    in_maps = pack_inputs(x1, x2, w, A, B, C)
    dev_in = runner.stage_inputs(in_maps)
    outs = runner.run(dev_in)
    return runner.unshard_out(outs)
